# revision 20
# baseline (speedup 1.0000x reference)
"""GATConv x2 + pools on 8 Trainium2 NeuronCores.

Sharding: one graph per core (edges are within-graph by construction:
src and dst share the same graph offset g*N), so no cross-core comms.

Per core, per GAT layer:
  dense phase : psum = h @ [W | W@Msrc | W@Mdst] (f16); xp|a_src rows go
                both to SBUF (xp_sb, PE-gather rhs) and DRAM (dma_gather
                table); a_dst column block stays in SBUF (adst_sb).
  edge phase  : edges of each 128-node dst tile are split by src tile:
                  src tile <  S_SPLIT -> "PE chunks": per-cell counts are
                    padded to the max over cores (SPMD-uniform program);
                    per 128-edge chunk, 2-4 accumulating matmuls gather
                    xp rows from xp_sb via host-built fp8 one-hot lhsT
                    (osrcT stream), contraction over the src tile.
                  src tile >= S_SPLIT -> "DMA chunks": bulk dma_gather of
                    table rows by src (SWDGE desc-gen on the Q7s is the
                    machine bottleneck, so only ~half the edges use it).
                dst-side one-hots (odst = scatter lhsT, odstT = a_dst
                expand lhsT) stream in as dense fp8 DMA for all chunks;
                a_dst per edge = odstT^T @ adst_tile (tiny PE matmuls);
                logits = lrelu(a_src+a_dst); w = exp(logits)
                (segment-max skipped: alpha is exactly invariant to the
                shift, and |logits| <~ 2 so exp is safe);
                msg = xp * w (w broadcast 65x: ACT expand-write for DMA
                chunks, DVE stride-0 broadcast mult for PE chunks);
                scatter: psum[128n, 260] += odst^T @ [msg|w].
  epilogue    : out = psum_msg * recip(psum_denom) + bias;
                h' = elu(out)+1 = max(out,0) + exp(min(out,0))
                (the +1 is corrected in downstream weights host-side);
                pool x = h'.pw + (pb - sum(pw)); layer1 also transposes h'
                into h1T (f16) for layer-2's dense matmul.

Self-contained: hardcodes shapes from the problem spec.
"""

import numpy as np

B, N, F_IN = 8, 4096, 128
E = 524288
H, C = 4, 64
D1 = H * C  # 256
P = 128
NTILES = N // P  # 32
TROW = 384  # f16 table row: [(xp_h+b_h|1)x4 (260) | a_src 4 | pad] = 768B
S_SPLIT = 17  # src tiles < S_SPLIT are PE-gathered, rest dma_gathered

_CACHE = {}
_SKIP = set()


def _preprocess_edges(edge_index):
    """Split each dst tile's edges into PE cells (src tile < S_SPLIT,
    per-cell counts padded to the core max so the program is uniform)
    and a DMA remainder. Emit per-core one-hot streams + gather indices
    and the core-independent program structure."""
    src_all = np.asarray(edge_index[0]).astype(np.int64)
    dst_all = np.asarray(edge_index[1]).astype(np.int64)
    g = dst_all // N
    loops = np.arange(N, dtype=np.int64)
    # per (core, tile): edge lists split by src cell
    pe_cell = {}   # (b, t, s) -> (srcloc[], dstloc[])
    dma_part = {}  # (b, t) -> (src[], dstloc[])
    cnt_cell = np.zeros((B, NTILES, S_SPLIT), np.int64)
    cnt_dma = np.zeros((B, NTILES), np.int64)
    for b in range(B):
        m = g == b
        s = np.concatenate([src_all[m] - b * N, loops])
        d = np.concatenate([dst_all[m] - b * N, loops])
        t_arr = d // P
        st = s // P
        for t in range(NTILES):
            mt = t_arr == t
            s_t, d_t = s[mt], d[mt] - t * P
            pe_m = st[mt] < S_SPLIT
            s_pe, d_pe = s_t[pe_m], d_t[pe_m]
            cells = s_pe // P
            for cs in range(S_SPLIT):
                mc = cells == cs
                pe_cell[(b, t, cs)] = (s_pe[mc] % P, d_pe[mc])
                cnt_cell[b, t, cs] = mc.sum()
            dma_part[(b, t)] = (s_t[~pe_m], d_t[~pe_m])
            cnt_dma[b, t] = (~pe_m).sum()

    M = cnt_cell.max(axis=0)  # [NTILES, S_SPLIT] padded cell sizes
    npe = -(-M.sum(axis=1) // P)  # PE chunks per tile
    ndma = np.maximum(1, -(-cnt_dma.max(axis=0) // P))  # DMA chunks per tile
    nct_all = npe + ndma
    TC_ALL = int(nct_all.sum())
    EPAD_DMA = int(ndma.sum()) * P

    # runs: per tile, per PE chunk, list of (global_run_idx, src_tile)
    runs = []
    run_cells = []  # flat: src tile per global run
    cell_base = np.zeros((NTILES, S_SPLIT), np.int64)
    for t in range(NTILES):
        pos = 0
        tr = [[] for _ in range(int(npe[t]))]
        for cs in range(S_SPLIT):
            cell_base[t, cs] = pos
            lo, hi = pos, pos + int(M[t, cs])
            c = lo // P
            while lo < hi:
                seg_end = min(hi, (c + 1) * P)
                tr[c].append((len(run_cells), cs))
                run_cells.append(cs)
                lo = seg_end
                c += 1
            pos = hi
        runs.append(tuple(tuple(x) for x in tr))
    NRUNS = len(run_cells)
    run_cells = np.array(run_cells, np.int64)

    # chunk start (in the odst/odstT stream) per tile: PE chunks then DMA
    c0_all = np.zeros(NTILES, np.int64)
    r0 = np.zeros(NTILES, np.int64)
    acc = 0
    racc = 0
    for t in range(NTILES):
        c0_all[t] = acc
        acc += int(nct_all[t])
        r0[t] = racc
        racc += sum(len(tr) for tr in runs[t])

    # run -> (tile, chunk, col base within chunk) for osrcT filling
    run_tile = np.zeros(NRUNS, np.int64)
    run_chunk_base = np.zeros(NRUNS, np.int64)  # position of chunk start
    for t in range(NTILES):
        for c, tr in enumerate(runs[t]):
            for ri, cs in tr:
                run_tile[ri] = t
                run_chunk_base[ri] = c * P

    import ml_dtypes
    F8 = ml_dtypes.float8_e4m3
    idx_src = np.zeros((B, EPAD_DMA), np.int16)
    osrcT = np.zeros((B, P, NRUNS * P), np.uint8)
    odst = np.zeros((B, P, TC_ALL * P), np.uint8)
    odstT = np.zeros((B, P, TC_ALL * P), np.uint8)
    one8 = np.ones((), F8).view(np.uint8)

    # map (t, cs) -> run index at each position (for osrcT column addressing)
    run_of = {}
    for t in range(NTILES):
        for c, tr in enumerate(runs[t]):
            for ri, cs in tr:
                run_of[(t, c, cs)] = ri

    for b in range(B):
        dma_pos = 0
        for t in range(NTILES):
            cbase = c0_all[t] * P
            # PE cells
            for cs in range(S_SPLIT):
                sl, dl = pe_cell[(b, t, cs)]
                k = np.arange(len(sl))
                pos = cell_base[t, cs] + k  # position within tile's PE space
                ch = pos // P
                e = pos % P
                ris = np.array([run_of[(t, int(c), cs)] for c in ch], np.int64) \
                    if len(sl) else np.zeros(0, np.int64)
                osrcT[b, sl, ris * P + e] = one8
                odst[b, e, cbase + ch * P + dl] = one8
                odstT[b, dl, cbase + ch * P + e] = one8
            # DMA part
            sd, dd = dma_part[(b, t)]
            k = np.arange(len(sd))
            ch = k // P
            e = k % P
            base = cbase + int(npe[t]) * P
            odst[b, e, base + ch * P + dd] = one8
            odstT[b, dd, base + ch * P + e] = one8
            L = int(ndma[t]) * P
            se = np.zeros(L, np.int64)
            se[: len(sd)] = sd - S_SPLIT * P
            idx_src[b, dma_pos : dma_pos + L] = se
            dma_pos += L

    def wrap(a):
        w = a.reshape(B, EPAD_DMA // 16, 16).transpose(0, 2, 1)
        return np.ascontiguousarray(np.tile(w, (1, 8, 1)))

    meta = {
        "npe": tuple(int(x) for x in npe),
        "ndma": tuple(int(x) for x in ndma),
        "runs": tuple(runs),
        "c0": tuple(int(x) for x in c0_all),
        "r0": tuple(int(x) for x in r0),
        "NRUNS": NRUNS,
        "TC_ALL": TC_ALL,
        "EPAD_DMA": EPAD_DMA,
    }
    return meta, wrap(idx_src), osrcT.view(F8), odst.view(F8), odstT.view(F8)


def _aug_w(W, att_s, att_d):
    """[ (W_h | 0) x4 heads | W@Msrc | W@Mdst ] -> [K, 268].
    The 65th column of each head block becomes a ones column (via the
    bias row), so the scatter rhs gets msg and denom from one scalar-mult."""
    K = W.shape[0]
    out = np.zeros((K, 268), np.float32)
    Msrc = np.zeros((D1, H), np.float32)
    Mdst = np.zeros((D1, H), np.float32)
    for h in range(H):
        out[:, h * 65 : h * 65 + C] = W[:, h * C : (h + 1) * C]
        Msrc[h * C : (h + 1) * C, h] = att_s[h]
        Mdst[h * C : (h + 1) * C, h] = att_d[h]
    out[:, 260:264] = W @ Msrc
    out[:, 264:268] = W @ Mdst
    return out


def _dma_gather_raw(nc, out_ap, in_ap, idxs_ap, num_idxs, elem_size, elem_step):
    """dma_gather with arbitrary elem_size (bytes read per row); the table
    pitch (elem_step) must still be a multiple of 256B. HW-validated."""
    from concourse import mybir as mb
    gp = nc.gpsimd
    dt_size = mb.dt.size(in_ap.dtype)
    stride_bytes = elem_step * dt_size
    assert stride_bytes % 256 == 0
    _in_ap = gp.lower_ap_dma(in_ap, for_custom_bir_dma=True)
    _idxs_ap = gp.lower_ap(idxs_ap)
    _out_ap = gp.lower_ap(out_ap)
    return gp.add_instruction(
        mb.InstDMAGatherAnt(
            name=nc.get_next_instruction_name(),
            ins=[*_in_ap, _idxs_ap, gp.lower_val_access(gp.to_reg(num_idxs))],
            outs=[_out_ap],
            transpose=False,
            num_idxs=num_idxs,
            elem_size=elem_size,
            stride_bytes_256=stride_bytes // 256,
            gen_mode=0,
            single_packet=False,
            queue_num=0,
            sbuf_tokens_per_rank=0,
            sbuf_free_dim_per_rank=0,
            sbuf_free_dim_pad_per_rank=0,
            sbuf_byte_offset=0,
        )
    )


def _build_program(meta, num_cores, n_nodes=N):
    import concourse.bass as bass
    import concourse.tile as tile
    from concourse import bacc, mybir
    from concourse.masks import make_identity

    F16, F32, I16 = mybir.dt.float16, mybir.dt.float32, mybir.dt.int16
    F8 = mybir.dt.float8e4
    AF = mybir.ActivationFunctionType
    OP = mybir.AluOpType
    ntiles = n_nodes // P
    npe, ndma, runs = meta["npe"], meta["ndma"], meta["runs"]
    c0_all, r0 = meta["c0"], meta["r0"]
    NRUNS, TC_ALL, EPAD_DMA = meta["NRUNS"], meta["TC_ALL"], meta["EPAD_DMA"]

    nc = bacc.Bacc(
        "TRN2", target_bir_lowering=False, debug=False, num_devices=num_cores
    )
    xT_d = nc.declare_dram_parameter("xT", [P, n_nodes], F16, isOutput=False)
    w1_d = nc.declare_dram_parameter("W1a", [F_IN, 268], F16, isOutput=False)
    w2_d = nc.declare_dram_parameter("W2a", [D1 + 1, 268], F16, isOutput=False)
    bp_d = nc.declare_dram_parameter("bp", [2, D1], F32, isOutput=False)
    pbe_d = nc.declare_dram_parameter("pbe", [1, 2], F32, isOutput=False)
    bc1_d = nc.declare_dram_parameter("bc1", [1, 268], F16, isOutput=False)
    isrc_d = nc.declare_dram_parameter("isrc", [P, EPAD_DMA // 16], I16,
                                       isOutput=False)
    osr_d = nc.declare_dram_parameter("osrcT", [P, NRUNS * P], F8, isOutput=False)
    od_d = nc.declare_dram_parameter("odst", [P, TC_ALL * P], F8, isOutput=False)
    odT_d = nc.declare_dram_parameter("odstT", [P, TC_ALL * P], F8, isOutput=False)
    out_d = nc.declare_dram_parameter("out", [3, n_nodes], F32, isOutput=True)
    tbl_rows = n_nodes - S_SPLIT * P
    table_d = [
        nc.dram_tensor("table1", [tbl_rows, TROW], F16),
        nc.dram_tensor("table2", [tbl_rows, TROW], F16),
    ]

    from contextlib import ExitStack

    with tile.TileContext(nc) as tc, ExitStack() as ctx:
        pp = ctx.enter_context(tc.tile_pool(name="persist", bufs=1))
        gpool = ctx.enter_context(tc.tile_pool(name="gather", bufs=6))
        sopool = ctx.enter_context(tc.tile_pool(name="odstrm", bufs=4))
        stpool = ctx.enter_context(tc.tile_pool(name="odTstrm", bufs=4))
        orpool = ctx.enter_context(tc.tile_pool(name="osrstrm", bufs=4))
        lpool = ctx.enter_context(tc.tile_pool(name="logits", bufs=4))
        rpool = ctx.enter_context(tc.tile_pool(name="rhs", bufs=10))
        hpool = ctx.enter_context(tc.tile_pool(name="hwork", bufs=2))
        spool = ctx.enter_context(tc.tile_pool(name="small", bufs=6))
        pacc = ctx.enter_context(tc.tile_pool(name="pacc", bufs=2, space="PSUM"))
        pdense = ctx.enter_context(tc.tile_pool(name="pdense", bufs=1, space="PSUM"))
        pab = ctx.enter_context(tc.tile_pool(name="pab", bufs=1, space="PSUM"))
        ptrans = ctx.enter_context(tc.tile_pool(name="ptrans", bufs=1, space="PSUM"))
        pgat = ctx.enter_context(tc.tile_pool(name="pgat", bufs=3, space="PSUM"))

        # ---- persistent loads & constants ----
        xT_sb = pp.tile([P, n_nodes], F16, tag="xT")
        nc.sync.dma_start(
            xT_sb[:, S_SPLIT * P :], xT_d[:, S_SPLIT * P :]
        )
        nc.sync.dma_start(
            xT_sb[:, 0 : S_SPLIT * P], xT_d[:, 0 : S_SPLIT * P]
        )
        w1_sb = pp.tile([F_IN, 268], F16, tag="w1")
        nc.sync.dma_start(w1_sb[:], w1_d[:])
        w2a_sb = pp.tile([P, 268], F16, tag="w2a")
        nc.sync.dma_start(w2a_sb[:], w2_d[0:P, :])
        w2b_sb = pp.tile([P, 268], F16, tag="w2b")
        nc.sync.dma_start(w2b_sb[:], w2_d[P : 2 * P, :])
        w2c_sb = pp.tile([1, 268], F16, tag="w2c")
        nc.sync.dma_start(w2c_sb[:], w2_d[2 * P : 2 * P + 1, :])
        bp_rows = []
        for r in range(2):
            rt = pp.tile([1, D1], F32, tag=f"bprow{r}")
            nc.sync.dma_start(rt[:], bp_d[r : r + 1, :])
            bp_rows.append(rt)
        pbe_sb = pp.tile([1, 2], F32, tag="pbe")
        nc.sync.dma_start(pbe_sb[:], pbe_d[:])
        bc1_sb = pp.tile([1, 268], F16, tag="bc1")
        nc.sync.dma_start(bc1_sb[:], bc1_d[:])
        isrc_sb = pp.tile([P, EPAD_DMA // 16], I16, tag="isrc")
        nc.sync.dma_start(isrc_sb[:], isrc_d[:])

        ident = pp.tile([P, P], F32, tag="ident")
        make_identity(nc, ident[:])
        ones1h = pp.tile([1, P], F16, tag="ones1h")
        nc.vector.memset(ones1h[:], 1.0)
        onesf = pp.tile([1, P], F32, tag="onesf")
        nc.vector.memset(onesf[:], 1.0)
        inv128 = pp.tile([P, 1], F16, tag="inv128")
        nc.vector.memset(inv128[:], 1.0 / F_IN)

        def bcast_row(row_ap, width, tag):
            ps = pdense.tile([P, width], F32, space="PSUM", tag="pdense")
            nc.tensor.matmul(ps[:], lhsT=onesf[:], rhs=row_ap, start=True, stop=True)
            t = pp.tile([P, width], F32, tag=tag)
            nc.vector.tensor_copy(t[:], ps[:])
            return t

        pw1_bc = bcast_row(bp_rows[0][:], D1, "pw1bc")
        pw2_bc = bcast_row(bp_rows[1][:], D1, "pw2bc")
        pbe_bc = bcast_row(pbe_sb[0:1, :], 2, "pbebc")

        h1T_sb = pp.tile([P, 2 * n_nodes], F16, tag="h1T")
        xp_sb = pp.tile([P, 2 * S_SPLIT * 264], F16, tag="xp")
        adst_sb = pp.tile([P, 2 * ntiles * 4], F16, tag="adst")
        x1_sb = pp.tile([P, ntiles], F32, tag="x1")
        x2_sb = pp.tile([P, ntiles], F32, tag="x2")
        x0_sb = pp.tile([1, n_nodes], F32, tag="x0")

        # ---- dense phase ----
        def dense(layer, t, headpool=None):
            pool = headpool if headpool is not None else pdense
            tg = "pacc" if headpool is not None else "pdense"
            ps = pool.tile([P, 268], F32, space="PSUM", tag=tg)
            if layer == 0:
                nc.tensor.matmul(
                    ps[:], lhsT=xT_sb[:, t * P : (t + 1) * P], rhs=w1_sb[:],
                    start=True, stop=False,
                )
                nc.tensor.matmul(
                    ps[:], lhsT=ones1h[:], rhs=bc1_sb[:], start=False, stop=True,
                )
            else:
                nc.tensor.matmul(
                    ps[:], lhsT=h1T_sb[:, t * P : t * P + P], rhs=w2a_sb[:],
                    start=True, stop=False,
                )
                nc.tensor.matmul(
                    ps[:], lhsT=h1T_sb[:, n_nodes + t * P : n_nodes + t * P + P],
                    rhs=w2b_sb[:], start=False, stop=False,
                )
                nc.tensor.matmul(
                    ps[:], lhsT=ones1h[:], rhs=w2c_sb[:], start=False, stop=True,
                )
            if t < S_SPLIT:
                # xp rows to SBUF only (PE-gather rhs); never dma_gathered
                xoff = (layer * S_SPLIT + t) * 264
                nc.scalar.copy(xp_sb[:, xoff : xoff + 264], ps[:, 0:264])
            else:
                stg = lpool.tile([P, 264], F16, tag="stg")
                nc.scalar.copy(stg[:], ps[:, 0:264])
                tr0 = (t - S_SPLIT) * P
                nc.sync.dma_start(
                    table_d[layer][tr0 : tr0 + P, 0:264], stg[:]
                )
            off = (layer * ntiles + t) * 4
            nc.scalar.copy(adst_sb[:, off : off + 4], ps[:, 264:268])

        # ---- edge phase for one dst tile ----
        def edge_tile(layer, t, d0, pw_bc, xcol):
            n_pe, n_dma = npe[t], ndma[t]
            nct = n_pe + n_dma
            cb = c0_all[t]
            nruns_t = len([1 for tr in runs[t] for _ in tr])
            aoff = (layer * ntiles + t) * 4
            # streams + gather first (DMA/GPSIMD prefetch)
            Ld = n_dma * P
            gb = gpool.tile([P, n_dma, 264], F16, tag="gb")
            _dma_gather_raw(
                nc, gb[:], table_d[layer][:, 0:264],
                isrc_sb[:, d0 * 8 : d0 * 8 + Ld // 16],
                Ld, 264, TROW,
            )
            od = sopool.tile([P, nct * P], F8, tag="od")
            nc.sync.dma_start(od[:], od_d[:, cb * P : (cb + nct) * P])
            odT = stpool.tile([P, nct * P], F8, tag="odT")
            nc.sync.dma_start(odT[:], odT_d[:, cb * P : (cb + nct) * P])
            ors = orpool.tile([P, nruns_t * P], F8, tag="ors")
            nc.sync.dma_start(
                ors[:], osr_d[:, r0[t] * P : (r0[t] + nruns_t) * P]
            )
            ps_acc_full = pacc.tile([P, 268], F32, space="PSUM", tag="pacc")
            ps_acc = ps_acc_full[:, 0:260]
            mm = 0
            # --- PE chunks ---
            rbase = r0[t]
            for c in range(n_pe):
                psg = pgat.tile([P, 268], F32, space="PSUM", tag="pgat")
                tr = runs[t][c]
                for i, (ri, cs) in enumerate(tr):
                    xoff = (layer * S_SPLIT + cs) * 264
                    nc.tensor.matmul(
                        psg[:, 0:264],
                        lhsT=ors[:, (ri - rbase) * P : (ri - rbase + 1) * P],
                        rhs=xp_sb[:, xoff : xoff + 264],
                        start=(i == 0), stop=False, skip_group_check=True,
                    )
                nc.tensor.matmul(
                    psg[:, 260:264], lhsT=odT[:, c * P : (c + 1) * P],
                    rhs=adst_sb[:, aoff : aoff + 4],
                    start=False, stop=True, skip_group_check=True,
                )
                lgc = spool.tile([P, 4], F32, tag="lgc")
                nc.scalar.activation(lgc[:], psg[:, 260:264], AF.Prelu, alpha=0.2)
                e4 = spool.tile([P, 4], F16, tag="e4")
                nc.scalar.activation(e4[:], lgc[:], AF.Exp)
                rh = rpool.tile([P, 260], F16, tag="rhpe")
                nc.vector.tensor_tensor(
                    rh[:].rearrange("p (a b) -> p a b", a=H),
                    psg[:, 0:260].rearrange("p (a b) -> p a b", a=H),
                    e4[:].unsqueeze(2).to_broadcast([P, H, 65]),
                    op=OP.mult,
                )
                nc.tensor.matmul(
                    ps_acc[:], lhsT=od[:, c * P : (c + 1) * P], rhs=rh[:],
                    start=(mm == 0), stop=(mm == nct - 1),
                )
                mm += 1
            # --- DMA chunks ---
            ps_ab = pab.tile([P, n_dma * 4], F32, space="PSUM", tag="pab")
            for j in range(n_dma):
                nc.tensor.matmul(
                    ps_ab[:, 4 * j : 4 * j + 4],
                    lhsT=odT[:, (n_pe + j) * P : (n_pe + j + 1) * P],
                    rhs=adst_sb[:, aoff : aoff + 4],
                    start=True, stop=True,
                )
            ab = lpool.tile([P, n_dma, 4], F16, tag="ab")
            nc.scalar.copy(ab[:].rearrange("p c a -> p (c a)"), ps_ab[:])
            lg = lpool.tile([P, n_dma, 4], F32, tag="lg")
            nc.vector.tensor_tensor(
                lg[:], gb[:, :, 260:264], ab[:], op=OP.add
            )
            lg2 = lpool.tile([P, n_dma, 4], F32, tag="lg2")
            nc.vector.scalar_tensor_tensor(
                lg2[:], lg[:], 0.2, lg[:], op0=OP.mult, op1=OP.max
            )
            ebs = lpool.tile([P, n_dma, 4], F16, tag="ebs")
            nc.scalar.activation(ebs[:], lg2[:], AF.Exp)
            rhs2 = None
            for j in range(n_dma):
                if j % 2 == 0:
                    rhs2 = rpool.tile([P, 2, 260], F16, tag="rhs")
                    jn = min(2, n_dma - j)
                    nc.vector.tensor_tensor(
                        rhs2[:, 0:jn, :].rearrange("p c (a b) -> p c a b", a=H),
                        gb[:, j : j + jn, 0:260].rearrange(
                            "p c (a b) -> p c a b", a=H),
                        ebs[:, j : j + jn, :].unsqueeze(3).to_broadcast(
                            [P, jn, 4, 65]),
                        op=OP.mult,
                    )
                nc.tensor.matmul(
                    ps_acc[:], lhsT=od[:, (n_pe + j) * P : (n_pe + j + 1) * P],
                    rhs=rhs2[:, j % 2, :],
                    start=(mm == 0), stop=(mm == nct - 1),
                )
                mm += 1
            # epilogue
            rec = spool.tile([P, 4], F32, tag="rec")
            nc.vector.reciprocal(
                rec[:], ps_acc[:].rearrange("p (a b) -> p a b", a=H)[:, :, C]
            )
            y = hpool.tile([P, D1], F32, tag="y")
            nc.vector.tensor_tensor(
                y[:].rearrange("p (a b) -> p a b", a=H),
                ps_acc[:].rearrange("p (a b) -> p a b", a=H)[:, :, 0:C],
                rec[:].unsqueeze(2).to_broadcast([P, 4, C]),
                op=OP.mult,
            )
            t1 = hpool.tile([P, D1], F32, tag="t1")
            nc.scalar.activation(t1[:], y[:], AF.Relu, scale=-1.0)
            t2 = hpool.tile([P, D1], F32, tag="t2")
            nc.scalar.activation(t2[:], t1[:], AF.Exp, scale=-1.0)
            hp = hpool.tile([P, D1], F32, tag="hp")
            nc.vector.scalar_tensor_tensor(
                hp[:], y[:], 0.0, t2[:], op0=OP.max, op1=OP.add
            )
            scr = hpool.tile([P, D1], F32, tag="scr")
            nc.vector.scalar_tensor_tensor(
                scr[:], hp[:], 1.0, pw_bc[:], op0=OP.mult, op1=OP.mult,
                accum_out=xcol,
            )
            if layer == 0:
                for fh in range(2):
                    pst = ptrans.tile([P, P], F32, space="PSUM", tag="ptrans")
                    nc.tensor.transpose(
                        pst[:], hp[:, fh * P : (fh + 1) * P], ident[:]
                    )
                    nc.scalar.copy(
                        h1T_sb[:, fh * n_nodes + t * P : fh * n_nodes + t * P + P],
                        pst[:],
                    )

        def assemble_x(x_sb, pbe_col, row):
            xa = spool.tile([P, ntiles], F32, tag="xa")
            nc.vector.tensor_scalar(
                xa[:], x_sb[:], pbe_bc[:, pbe_col : pbe_col + 1], None, OP.add
            )
            pst = ptrans.tile([ntiles, P], F32, space="PSUM", tag="ptrans")
            nc.tensor.transpose(pst[:], xa[:], ident[:])
            xo = spool.tile([ntiles, P], F32, tag="xo")
            nc.vector.tensor_copy(xo[:], pst[:])
            nc.sync.dma_start(
                out_d[row : row + 1, :].rearrange("a (b c) -> (a b) c", b=ntiles),
                xo[:],
            )

        # ---- layer 1 dense: table tiles first (gathers wait on them);
        # alternate psum pools (pacc is idle here) for a 2-deep pipeline ----
        for i, t in enumerate(range(S_SPLIT, ntiles)):
            dense(0, t, headpool=pacc if i % 2 else None)
        for i, t in enumerate(range(S_SPLIT)):
            dense(0, t, headpool=pacc if i % 2 else None)
        # ---- layer 1 edges, layer 2 dense interleaved per tile ----
        tile_order = list(range(S_SPLIT, ntiles)) + list(range(S_SPLIT))
        d0_of = []
        acc = 0
        for t in range(ntiles):
            d0_of.append(acc)
            acc += ndma[t]
        for t in tile_order:
            edge_tile(0, t, d0_of[t], pw1_bc, x1_sb[:, t : t + 1])
            dense(1, t)
        assemble_x(x1_sb, 0, 1)
        # ---- x0 = mean_f x (PE/ACT slack while layer 2 gathers run) ----
        for k in range(n_nodes // 512):
            ps = pdense.tile([1, 512], F32, space="PSUM", tag="pdense")
            nc.tensor.matmul(
                ps[:], lhsT=inv128[:], rhs=xT_sb[:, k * 512 : (k + 1) * 512],
                start=True, stop=True,
            )
            nc.scalar.copy(x0_sb[:, k * 512 : (k + 1) * 512], ps[:])
        nc.sync.dma_start(out_d[0:1, :], x0_sb[:])
        # ---- layer 2 edges ----
        for t in tile_order:
            edge_tile(1, t, d0_of[t], pw2_bc, x2_sb[:, t : t + 1])
        assemble_x(x2_sb, 1, 2)

    nc.compile()
    return nc


def _prepare_inputs(x, edge_index, W1, att_src1, att_dst1, b1, W2, att_src2,
                    att_dst2, b2, pw1, pb1, pw2, pb2):
    meta, isrc_w, osrcT, odst, odstT = _preprocess_edges(edge_index)
    W1a = _aug_w(np.asarray(W1, np.float32), np.asarray(att_src1, np.float32),
                 np.asarray(att_dst1, np.float32))
    W2a = _aug_w(np.asarray(W2, np.float32), np.asarray(att_src2, np.float32),
                 np.asarray(att_dst2, np.float32))
    W2corr = -W2a.sum(axis=0, keepdims=True)
    b2a = np.asarray(b2, np.float32)
    for h in range(H):
        W2corr[0, h * 65 : h * 65 + C] += b2a[h * C : (h + 1) * C]
        W2corr[0, h * 65 + C] = 1.0  # ones column
    W2aug = np.concatenate([W2a, W2corr], axis=0).astype(np.float16)
    pw1 = np.asarray(pw1, np.float32)
    pw2 = np.asarray(pw2, np.float32)
    bp = np.stack([pw1[:, 0], pw2[:, 0]]).astype(np.float32)
    pbe = np.array(
        [[float(pb1[0]) - float(pw1.sum()), float(pb2[0]) - float(pw2.sum())]],
        np.float32,
    )
    bc1 = np.zeros((1, 268), np.float32)
    b1a = np.asarray(b1, np.float32)
    for h in range(H):
        bc1[0, h * 65 : h * 65 + C] = b1a[h * C : (h + 1) * C]
        bc1[0, h * 65 + C] = 1.0  # ones column
    bc1 = bc1.astype(np.float16)
    x = np.asarray(x, np.float32)
    in_maps = []
    for b in range(B):
        in_maps.append({
            "xT": np.ascontiguousarray(x[b].T).astype(np.float16),
            "W1a": W1a.astype(np.float16),
            "W2a": W2aug,
            "bp": bp,
            "pbe": pbe,
            "bc1": bc1,
            "isrc": isrc_w[b],
            "osrcT": np.ascontiguousarray(osrcT[b]),
            "odst": np.ascontiguousarray(odst[b]),
            "odstT": np.ascontiguousarray(odstT[b]),
        })
    return meta, in_maps


_RUN_KWARGS = {}
_LAST_RESULT = None


def kernel(**inputs):
    global _LAST_RESULT
    from concourse.bass_utils import run_bass_kernel_spmd

    meta, in_maps = _prepare_inputs(**inputs)
    key = (meta["npe"], meta["ndma"], meta["runs"])
    if key not in _CACHE:
        _CACHE[key] = _build_program(meta, B)
    nc = _CACHE[key]
    res = run_bass_kernel_spmd(nc, in_maps, list(range(B)), **_RUN_KWARGS)
    _LAST_RESULT = res
    out = np.stack([res.results[b]["out"].reshape(3 * N) for b in range(B)])
    return out.astype(np.float32)


# revision 21
# speedup vs baseline: 1.0454x; 1.0454x over previous
"""GATConv x2 + pools on 8 Trainium2 NeuronCores.

Sharding: one graph per core (edges are within-graph by construction:
src and dst share the same graph offset g*N), so no cross-core comms.

Per core, per GAT layer:
  dense phase : psum = h @ [W | W@Msrc | W@Mdst] (f16); xp|a_src rows go
                both to SBUF (xp_sb, PE-gather rhs) and DRAM (dma_gather
                table); a_dst column block stays in SBUF (adst_sb).
  edge phase  : edges of each 128-node dst tile are split by src tile:
                  src tile <  S_SPLIT -> "PE chunks": per-cell counts are
                    padded to the max over cores (SPMD-uniform program);
                    per 128-edge chunk, 2-4 accumulating matmuls gather
                    xp rows from xp_sb via host-built fp8 one-hot lhsT
                    (osrcT stream), contraction over the src tile.
                  src tile >= S_SPLIT -> "DMA chunks": bulk dma_gather of
                    table rows by src (SWDGE desc-gen on the Q7s is the
                    machine bottleneck, so only ~half the edges use it).
                dst-side one-hots (odst = scatter lhsT, odstT = a_dst
                expand lhsT) stream in as dense fp8 DMA for all chunks;
                a_dst per edge = odstT^T @ adst_tile (tiny PE matmuls);
                logits = lrelu(a_src+a_dst); w = exp(logits)
                (segment-max skipped: alpha is exactly invariant to the
                shift, and |logits| <~ 2 so exp is safe);
                msg = xp * w (w broadcast 65x: ACT expand-write for DMA
                chunks, DVE stride-0 broadcast mult for PE chunks);
                scatter: psum[128n, 260] += odst^T @ [msg|w].
  epilogue    : out = psum_msg * recip(psum_denom) + bias;
                h' = elu(out)+1 = max(out,0) + exp(min(out,0))
                (the +1 is corrected in downstream weights host-side);
                pool x = h'.pw + (pb - sum(pw)); layer1 also transposes h'
                into h1T (f16) for layer-2's dense matmul.

Self-contained: hardcodes shapes from the problem spec.
"""

import numpy as np

B, N, F_IN = 8, 4096, 128
E = 524288
H, C = 4, 64
D1 = H * C  # 256
P = 128
NTILES = N // P  # 32
TROW = 384  # f16 table row: [(xp_h+b_h|1)x4 (260) | a_src 4 | pad] = 768B
S_SPLIT = 17  # src tiles < S_SPLIT are PE-gathered, rest dma_gathered

_CACHE = {}
_SKIP = set()


def _preprocess_edges(edge_index):
    """Split each dst tile's edges into PE cells (src tile < S_SPLIT,
    per-cell counts padded to the core max so the program is uniform)
    and a DMA remainder. Emit per-core one-hot streams + gather indices
    and the core-independent program structure."""
    src_all = np.asarray(edge_index[0]).astype(np.int64)
    dst_all = np.asarray(edge_index[1]).astype(np.int64)
    g = dst_all // N
    loops = np.arange(N, dtype=np.int64)
    # per (core, tile): edge lists split by src cell
    pe_cell = {}   # (b, t, s) -> (srcloc[], dstloc[])
    dma_part = {}  # (b, t) -> (src[], dstloc[])
    cnt_cell = np.zeros((B, NTILES, S_SPLIT), np.int64)
    cnt_dma = np.zeros((B, NTILES), np.int64)
    for b in range(B):
        m = g == b
        s = np.concatenate([src_all[m] - b * N, loops])
        d = np.concatenate([dst_all[m] - b * N, loops])
        t_arr = d // P
        st = s // P
        for t in range(NTILES):
            mt = t_arr == t
            s_t, d_t = s[mt], d[mt] - t * P
            pe_m = st[mt] < S_SPLIT
            s_pe, d_pe = s_t[pe_m], d_t[pe_m]
            cells = s_pe // P
            for cs in range(S_SPLIT):
                mc = cells == cs
                pe_cell[(b, t, cs)] = (s_pe[mc] % P, d_pe[mc])
                cnt_cell[b, t, cs] = mc.sum()
            dma_part[(b, t)] = (s_t[~pe_m], d_t[~pe_m])
            cnt_dma[b, t] = (~pe_m).sum()

    M = cnt_cell.max(axis=0)  # [NTILES, S_SPLIT] padded cell sizes
    npe = -(-M.sum(axis=1) // P)  # PE chunks per tile
    ndma = np.maximum(1, -(-cnt_dma.max(axis=0) // P))  # DMA chunks per tile
    nct_all = npe + ndma
    TC_ALL = int(nct_all.sum())
    EPAD_DMA = int(ndma.sum()) * P

    # runs: per tile, per PE chunk, list of (global_run_idx, src_tile)
    runs = []
    run_cells = []  # flat: src tile per global run
    cell_base = np.zeros((NTILES, S_SPLIT), np.int64)
    for t in range(NTILES):
        pos = 0
        tr = [[] for _ in range(int(npe[t]))]
        for cs in range(S_SPLIT):
            cell_base[t, cs] = pos
            lo, hi = pos, pos + int(M[t, cs])
            c = lo // P
            while lo < hi:
                seg_end = min(hi, (c + 1) * P)
                tr[c].append((len(run_cells), cs))
                run_cells.append(cs)
                lo = seg_end
                c += 1
            pos = hi
        runs.append(tuple(tuple(x) for x in tr))
    NRUNS = len(run_cells)
    run_cells = np.array(run_cells, np.int64)

    # chunk start (in the odst/odstT stream) per tile: PE chunks then DMA
    c0_all = np.zeros(NTILES, np.int64)
    r0 = np.zeros(NTILES, np.int64)
    acc = 0
    racc = 0
    for t in range(NTILES):
        c0_all[t] = acc
        acc += int(nct_all[t])
        r0[t] = racc
        racc += sum(len(tr) for tr in runs[t])

    # run -> (tile, chunk, col base within chunk) for osrcT filling
    run_tile = np.zeros(NRUNS, np.int64)
    run_chunk_base = np.zeros(NRUNS, np.int64)  # position of chunk start
    for t in range(NTILES):
        for c, tr in enumerate(runs[t]):
            for ri, cs in tr:
                run_tile[ri] = t
                run_chunk_base[ri] = c * P

    import ml_dtypes
    F8 = ml_dtypes.float8_e4m3
    idx_src = np.zeros((B, EPAD_DMA), np.int16)
    osrcT = np.zeros((B, P, NRUNS * P), np.uint8)
    odst = np.zeros((B, P, TC_ALL * P), np.uint8)
    odstT = np.zeros((B, P, TC_ALL * P), np.uint8)
    one8 = np.ones((), F8).view(np.uint8)

    # map (t, cs) -> run index at each position (for osrcT column addressing)
    run_of = {}
    for t in range(NTILES):
        for c, tr in enumerate(runs[t]):
            for ri, cs in tr:
                run_of[(t, c, cs)] = ri

    for b in range(B):
        dma_pos = 0
        for t in range(NTILES):
            cbase = c0_all[t] * P
            # PE cells
            for cs in range(S_SPLIT):
                sl, dl = pe_cell[(b, t, cs)]
                k = np.arange(len(sl))
                pos = cell_base[t, cs] + k  # position within tile's PE space
                ch = pos // P
                e = pos % P
                ris = np.array([run_of[(t, int(c), cs)] for c in ch], np.int64) \
                    if len(sl) else np.zeros(0, np.int64)
                osrcT[b, sl, ris * P + e] = one8
                odst[b, e, cbase + ch * P + dl] = one8
                odstT[b, dl, cbase + ch * P + e] = one8
            # DMA part
            sd, dd = dma_part[(b, t)]
            k = np.arange(len(sd))
            ch = k // P
            e = k % P
            base = cbase + int(npe[t]) * P
            odst[b, e, base + ch * P + dd] = one8
            odstT[b, dd, base + ch * P + e] = one8
            L = int(ndma[t]) * P
            se = np.zeros(L, np.int64)
            se[: len(sd)] = sd - S_SPLIT * P
            idx_src[b, dma_pos : dma_pos + L] = se
            dma_pos += L

    def wrap(a):
        w = a.reshape(B, EPAD_DMA // 16, 16).transpose(0, 2, 1)
        return np.ascontiguousarray(np.tile(w, (1, 8, 1)))

    meta = {
        "npe": tuple(int(x) for x in npe),
        "ndma": tuple(int(x) for x in ndma),
        "runs": tuple(runs),
        "c0": tuple(int(x) for x in c0_all),
        "r0": tuple(int(x) for x in r0),
        "NRUNS": NRUNS,
        "TC_ALL": TC_ALL,
        "EPAD_DMA": EPAD_DMA,
    }
    return meta, wrap(idx_src), osrcT.view(F8), odst.view(F8), odstT.view(F8)


def _aug_w(W, att_s, att_d):
    """[ (W_h | 0) x4 heads | W@Msrc | W@Mdst ] -> [K, 268].
    The 65th column of each head block becomes a ones column (via the
    bias row), so the scatter rhs gets msg and denom from one scalar-mult."""
    K = W.shape[0]
    out = np.zeros((K, 268), np.float32)
    Msrc = np.zeros((D1, H), np.float32)
    Mdst = np.zeros((D1, H), np.float32)
    for h in range(H):
        out[:, h * 65 : h * 65 + C] = W[:, h * C : (h + 1) * C]
        Msrc[h * C : (h + 1) * C, h] = att_s[h]
        Mdst[h * C : (h + 1) * C, h] = att_d[h]
    out[:, 260:264] = W @ Msrc
    out[:, 264:268] = W @ Mdst
    return out


def _dma_gather_raw(nc, out_ap, in_ap, idxs_ap, num_idxs, elem_size, elem_step):
    """dma_gather with arbitrary elem_size (bytes read per row); the table
    pitch (elem_step) must still be a multiple of 256B. HW-validated."""
    from concourse import mybir as mb
    gp = nc.gpsimd
    dt_size = mb.dt.size(in_ap.dtype)
    stride_bytes = elem_step * dt_size
    assert stride_bytes % 256 == 0
    _in_ap = gp.lower_ap_dma(in_ap, for_custom_bir_dma=True)
    _idxs_ap = gp.lower_ap(idxs_ap)
    _out_ap = gp.lower_ap(out_ap)
    return gp.add_instruction(
        mb.InstDMAGatherAnt(
            name=nc.get_next_instruction_name(),
            ins=[*_in_ap, _idxs_ap, gp.lower_val_access(gp.to_reg(num_idxs))],
            outs=[_out_ap],
            transpose=False,
            num_idxs=num_idxs,
            elem_size=elem_size,
            stride_bytes_256=stride_bytes // 256,
            gen_mode=0,
            single_packet=False,
            queue_num=0,
            sbuf_tokens_per_rank=0,
            sbuf_free_dim_per_rank=0,
            sbuf_free_dim_pad_per_rank=0,
            sbuf_byte_offset=0,
        )
    )


def _build_program(meta, num_cores, n_nodes=N):
    import concourse.bass as bass
    import concourse.tile as tile
    from concourse import bacc, mybir
    from concourse.masks import make_identity

    F16, F32, I16 = mybir.dt.float16, mybir.dt.float32, mybir.dt.int16
    F8 = mybir.dt.float8e4
    AF = mybir.ActivationFunctionType
    OP = mybir.AluOpType
    ntiles = n_nodes // P
    npe, ndma, runs = meta["npe"], meta["ndma"], meta["runs"]
    c0_all, r0 = meta["c0"], meta["r0"]
    NRUNS, TC_ALL, EPAD_DMA = meta["NRUNS"], meta["TC_ALL"], meta["EPAD_DMA"]

    nc = bacc.Bacc(
        "TRN2", target_bir_lowering=False, debug=False, num_devices=num_cores
    )
    xT_d = nc.declare_dram_parameter("xT", [P, n_nodes], F16, isOutput=False)
    w1_d = nc.declare_dram_parameter("W1a", [F_IN, 268], F16, isOutput=False)
    w2_d = nc.declare_dram_parameter("W2a", [D1 + 1, 268], F16, isOutput=False)
    bp_d = nc.declare_dram_parameter("bp", [2, D1], F32, isOutput=False)
    pbe_d = nc.declare_dram_parameter("pbe", [1, 2], F32, isOutput=False)
    bc1_d = nc.declare_dram_parameter("bc1", [1, 268], F16, isOutput=False)
    isrc_d = nc.declare_dram_parameter("isrc", [P, EPAD_DMA // 16], I16,
                                       isOutput=False)
    osr_d = nc.declare_dram_parameter("osrcT", [P, NRUNS * P], F8, isOutput=False)
    od_d = nc.declare_dram_parameter("odst", [P, TC_ALL * P], F8, isOutput=False)
    odT_d = nc.declare_dram_parameter("odstT", [P, TC_ALL * P], F8, isOutput=False)
    out_d = nc.declare_dram_parameter("out", [3, n_nodes], F32, isOutput=True)
    tbl_rows = n_nodes - S_SPLIT * P
    table_d = [
        nc.dram_tensor("table1", [tbl_rows, TROW], F16),
        nc.dram_tensor("table2", [tbl_rows, TROW], F16),
    ]

    from contextlib import ExitStack

    with tile.TileContext(nc) as tc, ExitStack() as ctx:
        pp = ctx.enter_context(tc.tile_pool(name="persist", bufs=1))
        gpool = ctx.enter_context(tc.tile_pool(name="gather", bufs=6))
        sopool = ctx.enter_context(tc.tile_pool(name="odstrm", bufs=4))
        stpool = ctx.enter_context(tc.tile_pool(name="odTstrm", bufs=4))
        orpool = ctx.enter_context(tc.tile_pool(name="osrstrm", bufs=4))
        lpool = ctx.enter_context(tc.tile_pool(name="logits", bufs=4))
        rpool = ctx.enter_context(tc.tile_pool(name="rhs", bufs=10))
        hpool = ctx.enter_context(tc.tile_pool(name="hwork", bufs=2))
        spool = ctx.enter_context(tc.tile_pool(name="small", bufs=6))
        pacc = ctx.enter_context(tc.tile_pool(name="pacc", bufs=2, space="PSUM"))
        pdense = ctx.enter_context(tc.tile_pool(name="pdense", bufs=1, space="PSUM"))
        pab = ctx.enter_context(tc.tile_pool(name="pab", bufs=1, space="PSUM"))
        ptrans = ctx.enter_context(tc.tile_pool(name="ptrans", bufs=1, space="PSUM"))
        pgat = ctx.enter_context(tc.tile_pool(name="pgat", bufs=3, space="PSUM"))

        # ---- persistent loads & constants ----
        xT_sb = pp.tile([P, n_nodes], F16, tag="xT")
        nc.sync.dma_start(
            xT_sb[:, S_SPLIT * P :], xT_d[:, S_SPLIT * P :]
        )
        nc.sync.dma_start(
            xT_sb[:, 0 : S_SPLIT * P], xT_d[:, 0 : S_SPLIT * P]
        )
        w1_sb = pp.tile([F_IN, 268], F16, tag="w1")
        nc.sync.dma_start(w1_sb[:], w1_d[:])
        w2a_sb = pp.tile([P, 268], F16, tag="w2a")
        nc.sync.dma_start(w2a_sb[:], w2_d[0:P, :])
        w2b_sb = pp.tile([P, 268], F16, tag="w2b")
        nc.sync.dma_start(w2b_sb[:], w2_d[P : 2 * P, :])
        w2c_sb = pp.tile([1, 268], F16, tag="w2c")
        nc.sync.dma_start(w2c_sb[:], w2_d[2 * P : 2 * P + 1, :])
        bp_rows = []
        for r in range(2):
            rt = pp.tile([1, D1], F32, tag=f"bprow{r}")
            nc.sync.dma_start(rt[:], bp_d[r : r + 1, :])
            bp_rows.append(rt)
        pbe_sb = pp.tile([1, 2], F32, tag="pbe")
        nc.sync.dma_start(pbe_sb[:], pbe_d[:])
        bc1_sb = pp.tile([1, 268], F16, tag="bc1")
        nc.sync.dma_start(bc1_sb[:], bc1_d[:])
        isrc_sb = pp.tile([P, EPAD_DMA // 16], I16, tag="isrc")
        nc.sync.dma_start(isrc_sb[:], isrc_d[:])

        ident = pp.tile([P, P], F32, tag="ident")
        make_identity(nc, ident[:])
        ones1h = pp.tile([1, P], F16, tag="ones1h")
        nc.vector.memset(ones1h[:], 1.0)
        onesf = pp.tile([1, P], F32, tag="onesf")
        nc.vector.memset(onesf[:], 1.0)
        inv128 = pp.tile([P, 1], F16, tag="inv128")
        nc.vector.memset(inv128[:], 1.0 / F_IN)

        def bcast_row(row_ap, width, tag):
            ps = pdense.tile([P, width], F32, space="PSUM", tag="pdense")
            nc.tensor.matmul(ps[:], lhsT=onesf[:], rhs=row_ap, start=True, stop=True)
            t = pp.tile([P, width], F32, tag=tag)
            nc.vector.tensor_copy(t[:], ps[:])
            return t

        pw1_bc = bcast_row(bp_rows[0][:], D1, "pw1bc")
        pw2_bc = bcast_row(bp_rows[1][:], D1, "pw2bc")
        pbe_bc = bcast_row(pbe_sb[0:1, :], 2, "pbebc")

        h1T_sb = pp.tile([P, 2 * n_nodes], F16, tag="h1T")
        xp_sb = pp.tile([P, 2 * S_SPLIT * 264], F16, tag="xp")
        adst_sb = pp.tile([P, 2 * ntiles * 4], F16, tag="adst")
        x1_sb = pp.tile([P, ntiles], F32, tag="x1")
        x2_sb = pp.tile([P, ntiles], F32, tag="x2")
        x0_sb = pp.tile([1, n_nodes], F32, tag="x0")

        # ---- dense phase ----
        def dense(layer, t, headpool=None):
            pool = headpool if headpool is not None else pdense
            tg = "pacc" if headpool is not None else "pdense"
            ps = pool.tile([P, 268], F32, space="PSUM", tag=tg)
            if layer == 0:
                nc.tensor.matmul(
                    ps[:], lhsT=xT_sb[:, t * P : (t + 1) * P], rhs=w1_sb[:],
                    start=True, stop=False,
                )
                nc.tensor.matmul(
                    ps[:], lhsT=ones1h[:], rhs=bc1_sb[:], start=False, stop=True,
                )
            else:
                nc.tensor.matmul(
                    ps[:], lhsT=h1T_sb[:, t * P : t * P + P], rhs=w2a_sb[:],
                    start=True, stop=False,
                )
                nc.tensor.matmul(
                    ps[:], lhsT=h1T_sb[:, n_nodes + t * P : n_nodes + t * P + P],
                    rhs=w2b_sb[:], start=False, stop=False,
                )
                nc.tensor.matmul(
                    ps[:], lhsT=ones1h[:], rhs=w2c_sb[:], start=False, stop=True,
                )
            if t < S_SPLIT:
                # xp rows to SBUF only (PE-gather rhs); never dma_gathered
                xoff = (layer * S_SPLIT + t) * 264
                nc.scalar.copy(xp_sb[:, xoff : xoff + 264], ps[:, 0:264])
            else:
                stg = lpool.tile([P, 264], F16, tag="stg")
                nc.scalar.copy(stg[:], ps[:, 0:264])
                tr0 = (t - S_SPLIT) * P
                nc.sync.dma_start(
                    table_d[layer][tr0 : tr0 + P, 0:264], stg[:]
                )
            off = (layer * ntiles + t) * 4
            nc.scalar.copy(adst_sb[:, off : off + 4], ps[:, 264:268])

        # ---- edge phase for one dst tile ----
        def edge_tile(layer, t, d0, pw_bc, xcol):
            n_pe, n_dma = npe[t], ndma[t]
            nct = n_pe + n_dma
            cb = c0_all[t]
            nruns_t = len([1 for tr in runs[t] for _ in tr])
            aoff = (layer * ntiles + t) * 4
            # streams + gather first (DMA/GPSIMD prefetch)
            gb = gpool.tile([P, n_dma, 264], F16, tag="gb")
            h_split = n_dma // 2
            for hh, (ja, jb) in enumerate([(0, h_split), (h_split, n_dma)]):
                Lh = (jb - ja) * P
                _dma_gather_raw(
                    nc, gb[:, ja:jb, :], table_d[layer][:, 0:264],
                    isrc_sb[:, (d0 + ja) * 8 : (d0 + ja) * 8 + Lh // 16],
                    Lh, 264, TROW,
                )
            od = sopool.tile([P, nct * P], F8, tag="od")
            nc.sync.dma_start(od[:], od_d[:, cb * P : (cb + nct) * P])
            odT = stpool.tile([P, nct * P], F8, tag="odT")
            nc.sync.dma_start(odT[:], odT_d[:, cb * P : (cb + nct) * P])
            ors = orpool.tile([P, nruns_t * P], F8, tag="ors")
            nc.sync.dma_start(
                ors[:], osr_d[:, r0[t] * P : (r0[t] + nruns_t) * P]
            )
            ps_acc_full = pacc.tile([P, 268], F32, space="PSUM", tag="pacc")
            ps_acc = ps_acc_full[:, 0:260]
            mm = 0
            # --- PE chunks ---
            rbase = r0[t]
            for c in range(n_pe):
                psg = pgat.tile([P, 268], F32, space="PSUM", tag="pgat")
                tr = runs[t][c]
                for i, (ri, cs) in enumerate(tr):
                    xoff = (layer * S_SPLIT + cs) * 264
                    nc.tensor.matmul(
                        psg[:, 0:264],
                        lhsT=ors[:, (ri - rbase) * P : (ri - rbase + 1) * P],
                        rhs=xp_sb[:, xoff : xoff + 264],
                        start=(i == 0), stop=False, skip_group_check=True,
                    )
                nc.tensor.matmul(
                    psg[:, 260:264], lhsT=odT[:, c * P : (c + 1) * P],
                    rhs=adst_sb[:, aoff : aoff + 4],
                    start=False, stop=True, skip_group_check=True,
                )
                lgc = spool.tile([P, 4], F32, tag="lgc")
                nc.scalar.activation(lgc[:], psg[:, 260:264], AF.Prelu, alpha=0.2)
                e4 = spool.tile([P, 4], F16, tag="e4")
                nc.scalar.activation(e4[:], lgc[:], AF.Exp)
                rh = rpool.tile([P, 260], F16, tag="rhpe")
                nc.vector.tensor_tensor(
                    rh[:].rearrange("p (a b) -> p a b", a=H),
                    psg[:, 0:260].rearrange("p (a b) -> p a b", a=H),
                    e4[:].unsqueeze(2).to_broadcast([P, H, 65]),
                    op=OP.mult,
                )
                nc.tensor.matmul(
                    ps_acc[:], lhsT=od[:, c * P : (c + 1) * P], rhs=rh[:],
                    start=(mm == 0), stop=(mm == nct - 1),
                )
                mm += 1
            # --- DMA chunks ---
            ps_ab = pab.tile([P, n_dma * 4], F32, space="PSUM", tag="pab")
            for j in range(n_dma):
                nc.tensor.matmul(
                    ps_ab[:, 4 * j : 4 * j + 4],
                    lhsT=odT[:, (n_pe + j) * P : (n_pe + j + 1) * P],
                    rhs=adst_sb[:, aoff : aoff + 4],
                    start=True, stop=True,
                )
            ab = lpool.tile([P, n_dma, 4], F16, tag="ab")
            nc.scalar.copy(ab[:].rearrange("p c a -> p (c a)"), ps_ab[:])
            ebs = lpool.tile([P, n_dma, 4], F16, tag="ebs")
            for (ja, jb) in [(0, h_split), (h_split, n_dma)]:
                lg = lpool.tile([P, n_dma, 4], F32, tag="lg")
                nc.vector.tensor_tensor(
                    lg[:, ja:jb], gb[:, ja:jb, 260:264], ab[:, ja:jb], op=OP.add
                )
                lg2 = lpool.tile([P, n_dma, 4], F32, tag="lg2")
                nc.vector.scalar_tensor_tensor(
                    lg2[:, ja:jb], lg[:, ja:jb], 0.2, lg[:, ja:jb],
                    op0=OP.mult, op1=OP.max
                )
                nc.scalar.activation(ebs[:, ja:jb], lg2[:, ja:jb], AF.Exp)
            rhs2 = None
            for j in range(n_dma):
                if j % 2 == 0:
                    rhs2 = rpool.tile([P, 2, 260], F16, tag="rhs")
                    jn = min(2, n_dma - j)
                    nc.vector.tensor_tensor(
                        rhs2[:, 0:jn, :].rearrange("p c (a b) -> p c a b", a=H),
                        gb[:, j : j + jn, 0:260].rearrange(
                            "p c (a b) -> p c a b", a=H),
                        ebs[:, j : j + jn, :].unsqueeze(3).to_broadcast(
                            [P, jn, 4, 65]),
                        op=OP.mult,
                    )
                nc.tensor.matmul(
                    ps_acc[:], lhsT=od[:, (n_pe + j) * P : (n_pe + j + 1) * P],
                    rhs=rhs2[:, j % 2, :],
                    start=(mm == 0), stop=(mm == nct - 1),
                )
                mm += 1
            # epilogue
            rec = spool.tile([P, 4], F32, tag="rec")
            nc.vector.reciprocal(
                rec[:], ps_acc[:].rearrange("p (a b) -> p a b", a=H)[:, :, C]
            )
            y = hpool.tile([P, D1], F32, tag="y")
            nc.vector.tensor_tensor(
                y[:].rearrange("p (a b) -> p a b", a=H),
                ps_acc[:].rearrange("p (a b) -> p a b", a=H)[:, :, 0:C],
                rec[:].unsqueeze(2).to_broadcast([P, 4, C]),
                op=OP.mult,
            )
            t1 = hpool.tile([P, D1], F32, tag="t1")
            nc.scalar.activation(t1[:], y[:], AF.Relu, scale=-1.0)
            t2 = hpool.tile([P, D1], F32, tag="t2")
            nc.scalar.activation(t2[:], t1[:], AF.Exp, scale=-1.0)
            hp = hpool.tile([P, D1], F32, tag="hp")
            nc.vector.scalar_tensor_tensor(
                hp[:], y[:], 0.0, t2[:], op0=OP.max, op1=OP.add
            )
            scr = hpool.tile([P, D1], F32, tag="scr")
            nc.vector.scalar_tensor_tensor(
                scr[:], hp[:], 1.0, pw_bc[:], op0=OP.mult, op1=OP.mult,
                accum_out=xcol,
            )
            if layer == 0:
                for fh in range(2):
                    pst = ptrans.tile([P, P], F32, space="PSUM", tag="ptrans")
                    nc.tensor.transpose(
                        pst[:], hp[:, fh * P : (fh + 1) * P], ident[:]
                    )
                    nc.scalar.copy(
                        h1T_sb[:, fh * n_nodes + t * P : fh * n_nodes + t * P + P],
                        pst[:],
                    )

        def assemble_x(x_sb, pbe_col, row):
            xa = spool.tile([P, ntiles], F32, tag="xa")
            nc.vector.tensor_scalar(
                xa[:], x_sb[:], pbe_bc[:, pbe_col : pbe_col + 1], None, OP.add
            )
            pst = ptrans.tile([ntiles, P], F32, space="PSUM", tag="ptrans")
            nc.tensor.transpose(pst[:], xa[:], ident[:])
            xo = spool.tile([ntiles, P], F32, tag="xo")
            nc.vector.tensor_copy(xo[:], pst[:])
            nc.sync.dma_start(
                out_d[row : row + 1, :].rearrange("a (b c) -> (a b) c", b=ntiles),
                xo[:],
            )

        # ---- layer 1 dense: table tiles first (gathers wait on them);
        # alternate psum pools (pacc is idle here) for a 2-deep pipeline ----
        for i, t in enumerate(range(S_SPLIT, ntiles)):
            dense(0, t, headpool=pacc if i % 2 else None)
        for i, t in enumerate(range(S_SPLIT)):
            dense(0, t, headpool=pacc if i % 2 else None)
        # ---- layer 1 edges, layer 2 dense interleaved per tile ----
        tile_order = list(range(S_SPLIT, ntiles)) + list(range(S_SPLIT))
        d0_of = []
        acc = 0
        for t in range(ntiles):
            d0_of.append(acc)
            acc += ndma[t]
        for t in tile_order:
            edge_tile(0, t, d0_of[t], pw1_bc, x1_sb[:, t : t + 1])
            dense(1, t)
        assemble_x(x1_sb, 0, 1)
        # ---- x0 = mean_f x (PE/ACT slack while layer 2 gathers run) ----
        for k in range(n_nodes // 512):
            ps = pdense.tile([1, 512], F32, space="PSUM", tag="pdense")
            nc.tensor.matmul(
                ps[:], lhsT=inv128[:], rhs=xT_sb[:, k * 512 : (k + 1) * 512],
                start=True, stop=True,
            )
            nc.scalar.copy(x0_sb[:, k * 512 : (k + 1) * 512], ps[:])
        nc.sync.dma_start(out_d[0:1, :], x0_sb[:])
        # ---- layer 2 edges ----
        for t in tile_order:
            edge_tile(1, t, d0_of[t], pw2_bc, x2_sb[:, t : t + 1])
        assemble_x(x2_sb, 1, 2)

    nc.compile()
    return nc


def _prepare_inputs(x, edge_index, W1, att_src1, att_dst1, b1, W2, att_src2,
                    att_dst2, b2, pw1, pb1, pw2, pb2):
    meta, isrc_w, osrcT, odst, odstT = _preprocess_edges(edge_index)
    W1a = _aug_w(np.asarray(W1, np.float32), np.asarray(att_src1, np.float32),
                 np.asarray(att_dst1, np.float32))
    W2a = _aug_w(np.asarray(W2, np.float32), np.asarray(att_src2, np.float32),
                 np.asarray(att_dst2, np.float32))
    W2corr = -W2a.sum(axis=0, keepdims=True)
    b2a = np.asarray(b2, np.float32)
    for h in range(H):
        W2corr[0, h * 65 : h * 65 + C] += b2a[h * C : (h + 1) * C]
        W2corr[0, h * 65 + C] = 1.0  # ones column
    W2aug = np.concatenate([W2a, W2corr], axis=0).astype(np.float16)
    pw1 = np.asarray(pw1, np.float32)
    pw2 = np.asarray(pw2, np.float32)
    bp = np.stack([pw1[:, 0], pw2[:, 0]]).astype(np.float32)
    pbe = np.array(
        [[float(pb1[0]) - float(pw1.sum()), float(pb2[0]) - float(pw2.sum())]],
        np.float32,
    )
    bc1 = np.zeros((1, 268), np.float32)
    b1a = np.asarray(b1, np.float32)
    for h in range(H):
        bc1[0, h * 65 : h * 65 + C] = b1a[h * C : (h + 1) * C]
        bc1[0, h * 65 + C] = 1.0  # ones column
    bc1 = bc1.astype(np.float16)
    x = np.asarray(x, np.float32)
    in_maps = []
    for b in range(B):
        in_maps.append({
            "xT": np.ascontiguousarray(x[b].T).astype(np.float16),
            "W1a": W1a.astype(np.float16),
            "W2a": W2aug,
            "bp": bp,
            "pbe": pbe,
            "bc1": bc1,
            "isrc": isrc_w[b],
            "osrcT": np.ascontiguousarray(osrcT[b]),
            "odst": np.ascontiguousarray(odst[b]),
            "odstT": np.ascontiguousarray(odstT[b]),
        })
    return meta, in_maps


_RUN_KWARGS = {}
_LAST_RESULT = None


def kernel(**inputs):
    global _LAST_RESULT
    from concourse.bass_utils import run_bass_kernel_spmd

    meta, in_maps = _prepare_inputs(**inputs)
    key = (meta["npe"], meta["ndma"], meta["runs"])
    if key not in _CACHE:
        _CACHE[key] = _build_program(meta, B)
    nc = _CACHE[key]
    res = run_bass_kernel_spmd(nc, in_maps, list(range(B)), **_RUN_KWARGS)
    _LAST_RESULT = res
    out = np.stack([res.results[b]["out"].reshape(3 * N) for b in range(B)])
    return out.astype(np.float32)


# revision 22
# speedup vs baseline: 1.0649x; 1.0187x over previous
"""GATConv x2 + pools on 8 Trainium2 NeuronCores.

Sharding: one graph per core (edges are within-graph by construction:
src and dst share the same graph offset g*N), so no cross-core comms.

Per core, per GAT layer:
  dense phase : psum = h @ [W | W@Msrc | W@Mdst] (f16); xp|a_src rows go
                both to SBUF (xp_sb, PE-gather rhs) and DRAM (dma_gather
                table); a_dst column block stays in SBUF (adst_sb).
  edge phase  : edges of each 128-node dst tile are split by src tile:
                  src tile <  S_SPLIT -> "PE chunks": per-cell counts are
                    padded to the max over cores (SPMD-uniform program);
                    per 128-edge chunk, 2-4 accumulating matmuls gather
                    xp rows from xp_sb via host-built fp8 one-hot lhsT
                    (osrcT stream), contraction over the src tile.
                  src tile >= S_SPLIT -> "DMA chunks": bulk dma_gather of
                    table rows by src (SWDGE desc-gen on the Q7s is the
                    machine bottleneck, so only ~half the edges use it).
                dst-side one-hots (odst = scatter lhsT, odstT = a_dst
                expand lhsT) stream in as dense fp8 DMA for all chunks;
                a_dst per edge = odstT^T @ adst_tile (tiny PE matmuls);
                logits = lrelu(a_src+a_dst); w = exp(logits)
                (segment-max skipped: alpha is exactly invariant to the
                shift, and |logits| <~ 2 so exp is safe);
                msg = xp * w (w broadcast 65x: ACT expand-write for DMA
                chunks, DVE stride-0 broadcast mult for PE chunks);
                scatter: psum[128n, 260] += odst^T @ [msg|w].
  epilogue    : out = psum_msg * recip(psum_denom) + bias;
                h' = elu(out)+1 = max(out,0) + exp(min(out,0))
                (the +1 is corrected in downstream weights host-side);
                pool x = h'.pw + (pb - sum(pw)); layer1 also transposes h'
                into h1T (f16) for layer-2's dense matmul.

Self-contained: hardcodes shapes from the problem spec.
"""

import numpy as np

B, N, F_IN = 8, 4096, 128
E = 524288
H, C = 4, 64
D1 = H * C  # 256
P = 128
NTILES = N // P  # 32
TROW = 384  # f16 table row: [(xp_h+b_h|1)x4 (260) | a_src 4 | pad] = 768B
S_SPLIT = 17  # src tiles < S_SPLIT are PE-gathered, rest dma_gathered

_CACHE = {}
_SKIP = set()


def _preprocess_edges(edge_index):
    """Split each dst tile's edges into PE cells (src tile < S_SPLIT,
    per-cell counts padded to the core max so the program is uniform)
    and a DMA remainder. Emit per-core one-hot streams + gather indices
    and the core-independent program structure."""
    src_all = np.asarray(edge_index[0]).astype(np.int64)
    dst_all = np.asarray(edge_index[1]).astype(np.int64)
    g = dst_all // N
    loops = np.arange(N, dtype=np.int64)
    # per (core, tile): edge lists split by src cell
    pe_cell = {}   # (b, t, s) -> (srcloc[], dstloc[])
    dma_part = {}  # (b, t) -> (src[], dstloc[])
    cnt_cell = np.zeros((B, NTILES, S_SPLIT), np.int64)
    cnt_dma = np.zeros((B, NTILES), np.int64)
    for b in range(B):
        m = g == b
        s = np.concatenate([src_all[m] - b * N, loops])
        d = np.concatenate([dst_all[m] - b * N, loops])
        t_arr = d // P
        st = s // P
        for t in range(NTILES):
            mt = t_arr == t
            s_t, d_t = s[mt], d[mt] - t * P
            pe_m = st[mt] < S_SPLIT
            s_pe, d_pe = s_t[pe_m], d_t[pe_m]
            cells = s_pe // P
            for cs in range(S_SPLIT):
                mc = cells == cs
                pe_cell[(b, t, cs)] = (s_pe[mc] % P, d_pe[mc])
                cnt_cell[b, t, cs] = mc.sum()
            dma_part[(b, t)] = (s_t[~pe_m], d_t[~pe_m])
            cnt_dma[b, t] = (~pe_m).sum()

    M = cnt_cell.max(axis=0)  # [NTILES, S_SPLIT] padded cell sizes
    npe = -(-M.sum(axis=1) // P)  # PE chunks per tile
    ndma = np.maximum(1, -(-cnt_dma.max(axis=0) // P))  # DMA chunks per tile
    nct_all = npe + ndma
    TC_ALL = int(nct_all.sum())
    EPAD_DMA = int(ndma.sum()) * P

    # runs: per tile, per PE chunk, list of (global_run_idx, src_tile)
    runs = []
    run_cells = []  # flat: src tile per global run
    cell_base = np.zeros((NTILES, S_SPLIT), np.int64)
    for t in range(NTILES):
        pos = 0
        tr = [[] for _ in range(int(npe[t]))]
        for cs in range(S_SPLIT):
            cell_base[t, cs] = pos
            lo, hi = pos, pos + int(M[t, cs])
            c = lo // P
            while lo < hi:
                seg_end = min(hi, (c + 1) * P)
                tr[c].append((len(run_cells), cs))
                run_cells.append(cs)
                lo = seg_end
                c += 1
            pos = hi
        runs.append(tuple(tuple(x) for x in tr))
    NRUNS = len(run_cells)
    run_cells = np.array(run_cells, np.int64)

    # chunk start (in the odst/odstT stream) per tile: PE chunks then DMA
    c0_all = np.zeros(NTILES, np.int64)
    r0 = np.zeros(NTILES, np.int64)
    acc = 0
    racc = 0
    for t in range(NTILES):
        c0_all[t] = acc
        acc += int(nct_all[t])
        r0[t] = racc
        racc += sum(len(tr) for tr in runs[t])

    # run -> (tile, chunk, col base within chunk) for osrcT filling
    run_tile = np.zeros(NRUNS, np.int64)
    run_chunk_base = np.zeros(NRUNS, np.int64)  # position of chunk start
    for t in range(NTILES):
        for c, tr in enumerate(runs[t]):
            for ri, cs in tr:
                run_tile[ri] = t
                run_chunk_base[ri] = c * P

    import ml_dtypes
    F8 = ml_dtypes.float8_e4m3
    idx_src = np.zeros((B, EPAD_DMA), np.int16)
    osrcT = np.zeros((B, P, NRUNS * P), np.uint8)
    odst = np.zeros((B, P, TC_ALL * P), np.uint8)
    odstT = np.zeros((B, P, TC_ALL * P), np.uint8)
    one8 = np.ones((), F8).view(np.uint8)

    # map (t, cs) -> run index at each position (for osrcT column addressing)
    run_of = {}
    for t in range(NTILES):
        for c, tr in enumerate(runs[t]):
            for ri, cs in tr:
                run_of[(t, c, cs)] = ri

    for b in range(B):
        dma_pos = 0
        for t in range(NTILES):
            cbase = c0_all[t] * P
            # PE cells
            for cs in range(S_SPLIT):
                sl, dl = pe_cell[(b, t, cs)]
                k = np.arange(len(sl))
                pos = cell_base[t, cs] + k  # position within tile's PE space
                ch = pos // P
                e = pos % P
                ris = np.array([run_of[(t, int(c), cs)] for c in ch], np.int64) \
                    if len(sl) else np.zeros(0, np.int64)
                osrcT[b, sl, ris * P + e] = one8
                odst[b, e, cbase + ch * P + dl] = one8
                odstT[b, dl, cbase + ch * P + e] = one8
            # DMA part
            sd, dd = dma_part[(b, t)]
            k = np.arange(len(sd))
            ch = k // P
            e = k % P
            base = cbase + int(npe[t]) * P
            odst[b, e, base + ch * P + dd] = one8
            odstT[b, dd, base + ch * P + e] = one8
            L = int(ndma[t]) * P
            se = np.zeros(L, np.int64)
            se[: len(sd)] = sd - S_SPLIT * P
            idx_src[b, dma_pos : dma_pos + L] = se
            dma_pos += L

    def wrap(a):
        w = a.reshape(B, EPAD_DMA // 16, 16).transpose(0, 2, 1)
        return np.ascontiguousarray(np.tile(w, (1, 8, 1)))

    meta = {
        "npe": tuple(int(x) for x in npe),
        "ndma": tuple(int(x) for x in ndma),
        "runs": tuple(runs),
        "c0": tuple(int(x) for x in c0_all),
        "r0": tuple(int(x) for x in r0),
        "NRUNS": NRUNS,
        "TC_ALL": TC_ALL,
        "EPAD_DMA": EPAD_DMA,
    }
    return meta, wrap(idx_src), osrcT.view(F8), odst.view(F8), odstT.view(F8)


def _aug_w(W, att_s, att_d):
    """[ (W_h | 0) x4 heads | W@Msrc | W@Mdst ] -> [K, 268].
    The 65th column of each head block becomes a ones column (via the
    bias row), so the scatter rhs gets msg and denom from one scalar-mult."""
    K = W.shape[0]
    out = np.zeros((K, 268), np.float32)
    Msrc = np.zeros((D1, H), np.float32)
    Mdst = np.zeros((D1, H), np.float32)
    for h in range(H):
        out[:, h * 65 : h * 65 + C] = W[:, h * C : (h + 1) * C]
        Msrc[h * C : (h + 1) * C, h] = att_s[h]
        Mdst[h * C : (h + 1) * C, h] = att_d[h]
    out[:, 260:264] = W @ Msrc
    out[:, 264:268] = W @ Mdst
    return out


def _dma_gather_raw(nc, out_ap, in_ap, idxs_ap, num_idxs, elem_size, elem_step):
    """dma_gather with arbitrary elem_size (bytes read per row); the table
    pitch (elem_step) must still be a multiple of 256B. HW-validated."""
    from concourse import mybir as mb
    gp = nc.gpsimd
    dt_size = mb.dt.size(in_ap.dtype)
    stride_bytes = elem_step * dt_size
    assert stride_bytes % 256 == 0
    _in_ap = gp.lower_ap_dma(in_ap, for_custom_bir_dma=True)
    _idxs_ap = gp.lower_ap(idxs_ap)
    _out_ap = gp.lower_ap(out_ap)
    return gp.add_instruction(
        mb.InstDMAGatherAnt(
            name=nc.get_next_instruction_name(),
            ins=[*_in_ap, _idxs_ap, gp.lower_val_access(gp.to_reg(num_idxs))],
            outs=[_out_ap],
            transpose=False,
            num_idxs=num_idxs,
            elem_size=elem_size,
            stride_bytes_256=stride_bytes // 256,
            gen_mode=0,
            single_packet=False,
            queue_num=0,
            sbuf_tokens_per_rank=0,
            sbuf_free_dim_per_rank=0,
            sbuf_free_dim_pad_per_rank=0,
            sbuf_byte_offset=0,
        )
    )


def _build_program(meta, num_cores, n_nodes=N):
    import concourse.bass as bass
    import concourse.tile as tile
    from concourse import bacc, mybir
    from concourse.masks import make_identity

    F16, F32, I16 = mybir.dt.float16, mybir.dt.float32, mybir.dt.int16
    F8 = mybir.dt.float8e4
    AF = mybir.ActivationFunctionType
    OP = mybir.AluOpType
    ntiles = n_nodes // P
    npe, ndma, runs = meta["npe"], meta["ndma"], meta["runs"]
    c0_all, r0 = meta["c0"], meta["r0"]
    NRUNS, TC_ALL, EPAD_DMA = meta["NRUNS"], meta["TC_ALL"], meta["EPAD_DMA"]

    nc = bacc.Bacc(
        "TRN2", target_bir_lowering=False, debug=False, num_devices=num_cores
    )
    xT_d = nc.declare_dram_parameter("xT", [P, n_nodes], F16, isOutput=False)
    w1_d = nc.declare_dram_parameter("W1a", [F_IN, 268], F16, isOutput=False)
    w2_d = nc.declare_dram_parameter("W2a", [D1 + 1, 268], F16, isOutput=False)
    bp_d = nc.declare_dram_parameter("bp", [2, D1], F32, isOutput=False)
    pbe_d = nc.declare_dram_parameter("pbe", [1, 2], F32, isOutput=False)
    bc1_d = nc.declare_dram_parameter("bc1", [1, 268], F16, isOutput=False)
    isrc_d = nc.declare_dram_parameter("isrc", [P, EPAD_DMA // 16], I16,
                                       isOutput=False)
    osr_d = nc.declare_dram_parameter("osrcT", [P, NRUNS * P], F8, isOutput=False)
    od_d = nc.declare_dram_parameter("odst", [P, TC_ALL * P], F8, isOutput=False)
    odT_d = nc.declare_dram_parameter("odstT", [P, TC_ALL * P], F8, isOutput=False)
    out_d = nc.declare_dram_parameter("out", [3, n_nodes], F32, isOutput=True)
    tbl_rows = n_nodes - S_SPLIT * P
    table_d = [
        nc.dram_tensor("table1", [tbl_rows, TROW], F16),
        nc.dram_tensor("table2", [tbl_rows, TROW], F16),
    ]

    from contextlib import ExitStack

    with tile.TileContext(nc) as tc, ExitStack() as ctx:
        pp = ctx.enter_context(tc.tile_pool(name="persist", bufs=1))
        gpool = ctx.enter_context(tc.tile_pool(name="gather", bufs=6))
        sopool = ctx.enter_context(tc.tile_pool(name="odstrm", bufs=4))
        stpool = ctx.enter_context(tc.tile_pool(name="odTstrm", bufs=4))
        orpool = ctx.enter_context(tc.tile_pool(name="osrstrm", bufs=4))
        lpool = ctx.enter_context(tc.tile_pool(name="logits", bufs=4))
        rpool = ctx.enter_context(tc.tile_pool(name="rhs", bufs=10))
        hpool = ctx.enter_context(tc.tile_pool(name="hwork", bufs=2))
        spool = ctx.enter_context(tc.tile_pool(name="small", bufs=6))
        pacc = ctx.enter_context(tc.tile_pool(name="pacc", bufs=2, space="PSUM"))
        pdense = ctx.enter_context(tc.tile_pool(name="pdense", bufs=1, space="PSUM"))
        pab = ctx.enter_context(tc.tile_pool(name="pab", bufs=1, space="PSUM"))
        ptrans = ctx.enter_context(tc.tile_pool(name="ptrans", bufs=1, space="PSUM"))
        pgat = ctx.enter_context(tc.tile_pool(name="pgat", bufs=3, space="PSUM"))

        # ---- persistent loads & constants ----
        xT_sb = pp.tile([P, n_nodes], F16, tag="xT")
        nc.sync.dma_start(
            xT_sb[:, S_SPLIT * P :], xT_d[:, S_SPLIT * P :]
        )
        nc.sync.dma_start(
            xT_sb[:, 0 : S_SPLIT * P], xT_d[:, 0 : S_SPLIT * P]
        )
        w1_sb = pp.tile([F_IN, 268], F16, tag="w1")
        nc.sync.dma_start(w1_sb[:], w1_d[:])
        w2a_sb = pp.tile([P, 268], F16, tag="w2a")
        nc.sync.dma_start(w2a_sb[:], w2_d[0:P, :])
        w2b_sb = pp.tile([P, 268], F16, tag="w2b")
        nc.sync.dma_start(w2b_sb[:], w2_d[P : 2 * P, :])
        w2c_sb = pp.tile([1, 268], F16, tag="w2c")
        nc.sync.dma_start(w2c_sb[:], w2_d[2 * P : 2 * P + 1, :])
        bp_rows = []
        for r in range(2):
            rt = pp.tile([1, D1], F32, tag=f"bprow{r}")
            nc.sync.dma_start(rt[:], bp_d[r : r + 1, :])
            bp_rows.append(rt)
        pbe_sb = pp.tile([1, 2], F32, tag="pbe")
        nc.sync.dma_start(pbe_sb[:], pbe_d[:])
        bc1_sb = pp.tile([1, 268], F16, tag="bc1")
        nc.sync.dma_start(bc1_sb[:], bc1_d[:])
        isrc_sb = pp.tile([P, EPAD_DMA // 16], I16, tag="isrc")
        nc.sync.dma_start(isrc_sb[:], isrc_d[:])

        ident = pp.tile([P, P], F32, tag="ident")
        make_identity(nc, ident[:])
        ones1h = pp.tile([1, P], F16, tag="ones1h")
        nc.vector.memset(ones1h[:], 1.0)
        onesf = pp.tile([1, P], F32, tag="onesf")
        nc.vector.memset(onesf[:], 1.0)
        inv128 = pp.tile([P, 1], F16, tag="inv128")
        nc.vector.memset(inv128[:], 1.0 / F_IN)

        def bcast_row(row_ap, width, tag):
            ps = pdense.tile([P, width], F32, space="PSUM", tag="pdense")
            nc.tensor.matmul(ps[:], lhsT=onesf[:], rhs=row_ap, start=True, stop=True)
            t = pp.tile([P, width], F32, tag=tag)
            nc.vector.tensor_copy(t[:], ps[:])
            return t

        pw1_bc = bcast_row(bp_rows[0][:], D1, "pw1bc")
        pw2_bc = bcast_row(bp_rows[1][:], D1, "pw2bc")
        pbe_bc = bcast_row(pbe_sb[0:1, :], 2, "pbebc")

        h1T_sb = pp.tile([P, 2 * n_nodes], F16, tag="h1T")
        xp_sb = pp.tile([P, 2 * S_SPLIT * 264], F16, tag="xp")
        adst_sb = pp.tile([P, 2 * ntiles * 4], F16, tag="adst")
        x1_sb = pp.tile([P, ntiles], F32, tag="x1")
        x2_sb = pp.tile([P, ntiles], F32, tag="x2")
        x0_sb = pp.tile([1, n_nodes], F32, tag="x0")

        # ---- dense phase ----
        def dense(layer, t, headpool=None):
            pool = headpool if headpool is not None else pdense
            tg = "pacc" if headpool is not None else "pdense"
            ps = pool.tile([P, 268], F32, space="PSUM", tag=tg)
            if layer == 0:
                nc.tensor.matmul(
                    ps[:], lhsT=xT_sb[:, t * P : (t + 1) * P], rhs=w1_sb[:],
                    start=True, stop=False,
                )
                nc.tensor.matmul(
                    ps[:], lhsT=ones1h[:], rhs=bc1_sb[:], start=False, stop=True,
                )
            else:
                nc.tensor.matmul(
                    ps[:], lhsT=h1T_sb[:, t * P : t * P + P], rhs=w2a_sb[:],
                    start=True, stop=False,
                )
                nc.tensor.matmul(
                    ps[:], lhsT=h1T_sb[:, n_nodes + t * P : n_nodes + t * P + P],
                    rhs=w2b_sb[:], start=False, stop=False,
                )
                nc.tensor.matmul(
                    ps[:], lhsT=ones1h[:], rhs=w2c_sb[:], start=False, stop=True,
                )
            if t < S_SPLIT:
                # xp rows to SBUF only (PE-gather rhs); never dma_gathered
                xoff = (layer * S_SPLIT + t) * 264
                nc.scalar.copy(xp_sb[:, xoff : xoff + 264], ps[:, 0:264])
            else:
                stg = lpool.tile([P, 264], F16, tag="stg")
                nc.scalar.copy(stg[:], ps[:, 0:264])
                tr0 = (t - S_SPLIT) * P
                nc.sync.dma_start(
                    table_d[layer][tr0 : tr0 + P, 0:264], stg[:]
                )
            off = (layer * ntiles + t) * 4
            nc.scalar.copy(adst_sb[:, off : off + 4], ps[:, 264:268])

        # ---- edge phase for one dst tile ----
        def edge_tile(layer, t, d0, pw_bc, xcol):
            n_pe, n_dma = npe[t], ndma[t]
            nct = n_pe + n_dma
            cb = c0_all[t]
            nruns_t = len([1 for tr in runs[t] for _ in tr])
            aoff = (layer * ntiles + t) * 4
            # streams + gather first (DMA/GPSIMD prefetch)
            Ld = n_dma * P
            gb = gpool.tile([P, n_dma, 264], F16, tag="gb")
            _dma_gather_raw(
                nc, gb[:], table_d[layer][:, 0:264],
                isrc_sb[:, d0 * 8 : d0 * 8 + Ld // 16],
                Ld, 264, TROW,
            )
            od = sopool.tile([P, nct * P], F8, tag="od")
            nc.sync.dma_start(od[:], od_d[:, cb * P : (cb + nct) * P])
            odT = stpool.tile([P, nct * P], F8, tag="odT")
            nc.sync.dma_start(odT[:], odT_d[:, cb * P : (cb + nct) * P])
            ors = orpool.tile([P, nruns_t * P], F8, tag="ors")
            nc.sync.dma_start(
                ors[:], osr_d[:, r0[t] * P : (r0[t] + nruns_t) * P]
            )
            ps_acc_full = pacc.tile([P, 268], F32, space="PSUM", tag="pacc")
            ps_acc = ps_acc_full[:, 0:260]
            mm = 0
            # --- PE chunks ---
            rbase = r0[t]
            for c in range(n_pe):
                psg = pgat.tile([P, 268], F32, space="PSUM", tag="pgat")
                tr = runs[t][c]
                for i, (ri, cs) in enumerate(tr):
                    xoff = (layer * S_SPLIT + cs) * 264
                    nc.tensor.matmul(
                        psg[:, 0:264],
                        lhsT=ors[:, (ri - rbase) * P : (ri - rbase + 1) * P],
                        rhs=xp_sb[:, xoff : xoff + 264],
                        start=(i == 0), stop=False, skip_group_check=True,
                    )
                nc.tensor.matmul(
                    psg[:, 260:264], lhsT=odT[:, c * P : (c + 1) * P],
                    rhs=adst_sb[:, aoff : aoff + 4],
                    start=False, stop=True, skip_group_check=True,
                )
                lgc = spool.tile([P, 4], F32, tag="lgc")
                nc.scalar.activation(lgc[:], psg[:, 260:264], AF.Prelu, alpha=0.2)
                e4 = spool.tile([P, 4], F16, tag="e4")
                nc.scalar.activation(e4[:], lgc[:], AF.Exp)
                rh = rpool.tile([P, 260], F16, tag="rhpe")
                nc.vector.tensor_tensor(
                    rh[:].rearrange("p (a b) -> p a b", a=H),
                    psg[:, 0:260].rearrange("p (a b) -> p a b", a=H),
                    e4[:].unsqueeze(2).to_broadcast([P, H, 65]),
                    op=OP.mult,
                )
                nc.tensor.matmul(
                    ps_acc[:], lhsT=od[:, c * P : (c + 1) * P], rhs=rh[:],
                    start=(mm == 0), stop=(mm == nct - 1),
                )
                mm += 1
            # --- DMA chunks ---
            ps_ab = pab.tile([P, n_dma * 4], F32, space="PSUM", tag="pab")
            for j in range(n_dma):
                nc.tensor.matmul(
                    ps_ab[:, 4 * j : 4 * j + 4],
                    lhsT=odT[:, (n_pe + j) * P : (n_pe + j + 1) * P],
                    rhs=adst_sb[:, aoff : aoff + 4],
                    start=True, stop=True,
                )
            ab = lpool.tile([P, n_dma, 4], F16, tag="ab")
            nc.scalar.copy(ab[:].rearrange("p c a -> p (c a)"), ps_ab[:])
            lg = lpool.tile([P, n_dma, 4], F32, tag="lg")
            nc.vector.tensor_tensor(
                lg[:], gb[:, :, 260:264], ab[:], op=OP.add
            )
            lg2 = lpool.tile([P, n_dma, 4], F32, tag="lg2")
            nc.scalar.activation(lg2[:], lg[:], AF.Prelu, alpha=0.2)
            ebs = lpool.tile([P, n_dma, 4], F16, tag="ebs")
            nc.scalar.activation(ebs[:], lg2[:], AF.Exp)
            rhs2 = None
            for j in range(n_dma):
                if j % 2 == 0:
                    rhs2 = rpool.tile([P, 2, 260], F16, tag="rhs")
                    jn = min(2, n_dma - j)
                    nc.vector.tensor_tensor(
                        rhs2[:, 0:jn, :].rearrange("p c (a b) -> p c a b", a=H),
                        gb[:, j : j + jn, 0:260].rearrange(
                            "p c (a b) -> p c a b", a=H),
                        ebs[:, j : j + jn, :].unsqueeze(3).to_broadcast(
                            [P, jn, 4, 65]),
                        op=OP.mult,
                    )
                nc.tensor.matmul(
                    ps_acc[:], lhsT=od[:, (n_pe + j) * P : (n_pe + j + 1) * P],
                    rhs=rhs2[:, j % 2, :],
                    start=(mm == 0), stop=(mm == nct - 1),
                )
                mm += 1
            # epilogue
            rec = spool.tile([P, 4], F32, tag="rec")
            nc.vector.reciprocal(
                rec[:], ps_acc[:].rearrange("p (a b) -> p a b", a=H)[:, :, C]
            )
            y = hpool.tile([P, D1], F32, tag="y")
            nc.vector.tensor_tensor(
                y[:].rearrange("p (a b) -> p a b", a=H),
                ps_acc[:].rearrange("p (a b) -> p a b", a=H)[:, :, 0:C],
                rec[:].unsqueeze(2).to_broadcast([P, 4, C]),
                op=OP.mult,
            )
            t1 = hpool.tile([P, D1], F32, tag="t1")
            nc.scalar.activation(t1[:], y[:], AF.Relu, scale=-1.0)
            t2 = hpool.tile([P, D1], F32, tag="t2")
            nc.scalar.activation(t2[:], t1[:], AF.Exp, scale=-1.0)
            hp = hpool.tile([P, D1], F32, tag="hp")
            nc.vector.scalar_tensor_tensor(
                hp[:], y[:], 0.0, t2[:], op0=OP.max, op1=OP.add
            )
            scr = hpool.tile([P, D1], F32, tag="scr")
            nc.vector.scalar_tensor_tensor(
                scr[:], hp[:], 1.0, pw_bc[:], op0=OP.mult, op1=OP.mult,
                accum_out=xcol,
            )
            if layer == 0:
                for fh in range(2):
                    pst = ptrans.tile([P, P], F32, space="PSUM", tag="ptrans")
                    nc.tensor.transpose(
                        pst[:], hp[:, fh * P : (fh + 1) * P], ident[:]
                    )
                    nc.scalar.copy(
                        h1T_sb[:, fh * n_nodes + t * P : fh * n_nodes + t * P + P],
                        pst[:],
                    )

        def assemble_x(x_sb, pbe_col, row):
            xa = spool.tile([P, ntiles], F32, tag="xa")
            nc.vector.tensor_scalar(
                xa[:], x_sb[:], pbe_bc[:, pbe_col : pbe_col + 1], None, OP.add
            )
            pst = ptrans.tile([ntiles, P], F32, space="PSUM", tag="ptrans")
            nc.tensor.transpose(pst[:], xa[:], ident[:])
            xo = spool.tile([ntiles, P], F32, tag="xo")
            nc.vector.tensor_copy(xo[:], pst[:])
            nc.sync.dma_start(
                out_d[row : row + 1, :].rearrange("a (b c) -> (a b) c", b=ntiles),
                xo[:],
            )

        # ---- layer 1 dense: table tiles first (gathers wait on them);
        # alternate psum pools (pacc is idle here) for a 2-deep pipeline ----
        for i, t in enumerate(range(S_SPLIT, ntiles)):
            dense(0, t, headpool=pacc if i % 2 else None)
        for i, t in enumerate(range(S_SPLIT)):
            dense(0, t, headpool=pacc if i % 2 else None)
        # ---- layer 1 edges, layer 2 dense interleaved per tile ----
        tile_order = list(range(S_SPLIT, ntiles)) + list(range(S_SPLIT))
        d0_of = []
        acc = 0
        for t in range(ntiles):
            d0_of.append(acc)
            acc += ndma[t]
        for t in tile_order:
            edge_tile(0, t, d0_of[t], pw1_bc, x1_sb[:, t : t + 1])
            dense(1, t)
        assemble_x(x1_sb, 0, 1)
        # ---- x0 = mean_f x (PE/ACT slack while layer 2 gathers run) ----
        for k in range(n_nodes // 512):
            ps = pdense.tile([1, 512], F32, space="PSUM", tag="pdense")
            nc.tensor.matmul(
                ps[:], lhsT=inv128[:], rhs=xT_sb[:, k * 512 : (k + 1) * 512],
                start=True, stop=True,
            )
            nc.scalar.copy(x0_sb[:, k * 512 : (k + 1) * 512], ps[:])
        nc.sync.dma_start(out_d[0:1, :], x0_sb[:])
        # ---- layer 2 edges ----
        for t in tile_order:
            edge_tile(1, t, d0_of[t], pw2_bc, x2_sb[:, t : t + 1])
        assemble_x(x2_sb, 1, 2)

    nc.compile()
    return nc


def _prepare_inputs(x, edge_index, W1, att_src1, att_dst1, b1, W2, att_src2,
                    att_dst2, b2, pw1, pb1, pw2, pb2):
    meta, isrc_w, osrcT, odst, odstT = _preprocess_edges(edge_index)
    W1a = _aug_w(np.asarray(W1, np.float32), np.asarray(att_src1, np.float32),
                 np.asarray(att_dst1, np.float32))
    W2a = _aug_w(np.asarray(W2, np.float32), np.asarray(att_src2, np.float32),
                 np.asarray(att_dst2, np.float32))
    W2corr = -W2a.sum(axis=0, keepdims=True)
    b2a = np.asarray(b2, np.float32)
    for h in range(H):
        W2corr[0, h * 65 : h * 65 + C] += b2a[h * C : (h + 1) * C]
        W2corr[0, h * 65 + C] = 1.0  # ones column
    W2aug = np.concatenate([W2a, W2corr], axis=0).astype(np.float16)
    pw1 = np.asarray(pw1, np.float32)
    pw2 = np.asarray(pw2, np.float32)
    bp = np.stack([pw1[:, 0], pw2[:, 0]]).astype(np.float32)
    pbe = np.array(
        [[float(pb1[0]) - float(pw1.sum()), float(pb2[0]) - float(pw2.sum())]],
        np.float32,
    )
    bc1 = np.zeros((1, 268), np.float32)
    b1a = np.asarray(b1, np.float32)
    for h in range(H):
        bc1[0, h * 65 : h * 65 + C] = b1a[h * C : (h + 1) * C]
        bc1[0, h * 65 + C] = 1.0  # ones column
    bc1 = bc1.astype(np.float16)
    x = np.asarray(x, np.float32)
    in_maps = []
    for b in range(B):
        in_maps.append({
            "xT": np.ascontiguousarray(x[b].T).astype(np.float16),
            "W1a": W1a.astype(np.float16),
            "W2a": W2aug,
            "bp": bp,
            "pbe": pbe,
            "bc1": bc1,
            "isrc": isrc_w[b],
            "osrcT": np.ascontiguousarray(osrcT[b]),
            "odst": np.ascontiguousarray(odst[b]),
            "odstT": np.ascontiguousarray(odstT[b]),
        })
    return meta, in_maps


_RUN_KWARGS = {}
_LAST_RESULT = None


def kernel(**inputs):
    global _LAST_RESULT
    from concourse.bass_utils import run_bass_kernel_spmd

    meta, in_maps = _prepare_inputs(**inputs)
    key = (meta["npe"], meta["ndma"], meta["runs"])
    if key not in _CACHE:
        _CACHE[key] = _build_program(meta, B)
    nc = _CACHE[key]
    res = run_bass_kernel_spmd(nc, in_maps, list(range(B)), **_RUN_KWARGS)
    _LAST_RESULT = res
    out = np.stack([res.results[b]["out"].reshape(3 * N) for b in range(B)])
    return out.astype(np.float32)
